# revision 6
# baseline (speedup 1.0000x reference)
"""CoPEGate Trainium2 kernel.

Computes out[b,h,t,s] = sigmoid((Q K^T)[b,h,t,s] / sqrt(D)) * (P P^T)[t,s] / sqrt(D)
for B=2, H=12, T=2048, D=64 (fp32 in/out), distributed over 8 NeuronCores.

Sharding: the 24 (b,h) pairs are split 3-per-core (head-parallel); the
positional matrix P is replicated and its T x T bias is computed on every
core (reused across that core's 3 heads). No cross-device communication.

Two levers vs the fp32 baseline (163 us, output-DMA bound):

1. fp16 output. The harness tolerance is rel-err 2e-2 (L2); writing the
   output as fp16 (adds ~3e-4 L2 rounding, upcast on host) halves output
   DMA from 48 to 24 MiB/core, moving the bound to the compute engines.

2. K=128 matmuls. HW-measured: a K=64 [64x128]@[64x512] fp16 chunk
   matmul streams at 427 ns (the PE clock governor holds 1.2 GHz for
   half-array work), while K=128 runs at 216 ns (2.4 GHz). So all
   stationary operands are zero-padded to 128 contraction rows (the zero
   rows multiply whatever sits in the moving tile's other half and add
   exactly 0): per head the lhsT is [q_h ; 0] or [0 ; q_h], and the
   moving tiles pack two real operands ([k0;k1] and [p;k2]) so no moving
   bandwidth is wasted. Zero halves are GPSIMD-memset once at startup.
   PE per-core drops from ~109 us to ~55 us.

Steady-state per row-tile (16 tiles of 128 rows), all stripes full-width
[128, 2048] f32 in PSUM (4 banks; two explicit pools ping-pong all 8):
  PE : s1 -> B, s0 -> A, pos(it+1) -> A, s2 -> B   (4x 512-col chunks each)
  ACT: sig1, sig0, sig2            (~1.96 us each, the ~95 us bound)
  DVE: mul1, mul0, cast', mul2     (fp16 muls in 2x mode ~1.3 us,
                                    pos cast f32->f16 ~2.4 us)
  DMA: 3x 512 KiB output stripes
The pos stripe for tile it+1 is matmul'd+cast during tile it so its PSUM
residency hides under sig0/sig2. Two of 16 casts run on ACT instead of
DVE to balance total engine time (ACT 95 us / DVE 96 us). GPSIMD is used
only for the startup memsets: its tensor ops share SBUF ports with DVE
and a concurrent GPSIMD multiply was measured to slow DVE tensor ops 7x.

Engine budget per core: ACT ~95 us, DVE ~96 us, PE ~55 us, DMA ~27 MiB
~80 us. Precision: q/k fp16, pos fp16 (pre-scaled by D**-0.25 on host),
output fp16; rel err ~5e-4 vs the 2e-2 gate.
"""

import math
import os
import sys

import numpy as np

sys.path.insert(0, "/opt/trn_rl_repo")

B, H, T, D = 2, 12, 2048, 64
N_CORES = 8
HPC = (B * H) // N_CORES  # heads per core
PT = 128  # output row-tile height (SBUF/PSUM partitions)
NT = T // PT  # row tiles
NCHUNK = 512  # matmul moving-operand free dim (one PSUM bank of fp32)
NCH = T // NCHUNK
INV_SQRT_D = 1.0 / math.sqrt(D)

ACT_CAST_TILES = (6, 12)  # tiles whose pos cast runs on ACT (engine balance)

_NC_CACHE = {}


def _build_nc():
    import concourse.bass as bass
    from concourse import bacc, mybir, tile

    f32 = mybir.dt.float32
    f16 = mybir.dt.float16
    Sigmoid = mybir.ActivationFunctionType.Sigmoid
    Copy = mybir.ActivationFunctionType.Copy

    nc = bacc.Bacc("TRN2", target_bir_lowering=False)

    qT = nc.dram_tensor("qT", [HPC, D, T], f16, kind="ExternalInput")
    kT = nc.dram_tensor("kT", [HPC, D, T], f16, kind="ExternalInput")
    # pos scale 1/sqrt(D) is folded into pT on the host (split across both
    # factors), so the pos matmul lands pre-scaled in PSUM.
    pT = nc.dram_tensor("pT", [D, T], f16, kind="ExternalInput")
    out = nc.dram_tensor("out", [HPC, T, T], f16, kind="ExternalOutput")

    with tile.TileContext(nc) as tc:
        with tc.tile_pool(name="ins", bufs=1) as ins_pool, \
             tc.tile_pool(name="pos", bufs=3) as pos_pool, \
             tc.tile_pool(name="gate", bufs=6) as gate_pool, \
             tc.tile_pool(name="outs", bufs=12) as outs_pool, \
             tc.tile_pool(name="psA", bufs=1, space="PSUM") as psA, \
             tc.tile_pool(name="psB", bufs=1, space="PSUM") as psB:

            # ---- inputs -----------------------------------------------
            # Moving (rhs) tiles, both halves real data:
            #   k01[j]: rows 0-63 = k0, rows 64-127 = k1
            #   pk2[j]: rows 0-63 = p (pre-scaled), rows 64-127 = k2
            # Stationary (lhsT) sets, zero-padded to K=128:
            #   qz[0] = [q0 ; 0]   (pairs with k01)
            #   qz[1] = [0 ; q1]   (pairs with k01)
            #   qz[2] = [0 ; q2]   (pairs with pk2)
            #   qz[3] = [p ; 0]    (pairs with pk2)
            # Zero halves are memset on GPSIMD (idle at startup) before
            # any matmul touches the set.
            kT01 = kT[0:2].rearrange("h d t -> (h d) t")
            k01_c, pk2_c = [], []
            qz_c = [[], [], [], []]
            Z = {0: (D, 2 * D), 1: (0, D), 2: (0, D), 3: (D, 2 * D)}
            for j in range(NCH):
                jsl = bass.ts(j, NCHUNK)
                for s in (1, 3, 0, 2):
                    qt = ins_pool.tile([2 * D, NCHUNK], f16, tag=f"qz{s}_{j}")
                    nc.gpsimd.memset(qt[Z[s][0] : Z[s][1], :], 0.0)
                    qz_c[s].append(qt)
                kc = ins_pool.tile([2 * D, NCHUNK], f16, tag=f"k01_{j}")
                nc.sync.dma_start(out=kc, in_=kT01[:, jsl])
                k01_c.append(kc)
                pc = ins_pool.tile([2 * D, NCHUNK], f16, tag=f"pk2_{j}")
                nc.sync.dma_start(out=pc[0:D, :], in_=pT[:, jsl])
                nc.sync.dma_start(out=pc[D : 2 * D, :], in_=kT[2][:, jsl])
                pk2_c.append(pc)
                nc.sync.dma_start(out=qz_c[3][j][0:D, :], in_=pT[:, jsl])
                nc.sync.dma_start(
                    out=qz_c[1][j][D : 2 * D, :], in_=qT[1][:, jsl]
                )
                nc.sync.dma_start(out=qz_c[0][j][0:D, :], in_=qT[0][:, jsl])
                nc.sync.dma_start(
                    out=qz_c[2][j][D : 2 * D, :], in_=qT[2][:, jsl]
                )

            def lhsT(s, it):
                # [128, 128] stationary block: columns it%4 of chunk it//4.
                sl = bass.ts(it % (NCHUNK // PT), PT)
                return qz_c[s][it // (NCHUNK // PT)][:, sl]

            def mm_stripe(psum, s, it):
                rhs_c = pk2_c if s in (2, 3) else k01_c
                for j in range(NCH):
                    nc.tensor.matmul(
                        psum[:, bass.ts(j, NCHUNK)],
                        lhsT(s, it),
                        rhs_c[j][:, :],
                        start=True,
                        stop=True,
                    )

            def head(h, it, pos_sb, pool):
                sp = pool.tile([PT, T], f32, tag="ps")
                mm_stripe(sp, h, it)
                gate = gate_pool.tile([PT, T], f16, tag="gate")
                nc.scalar.activation(gate, sp, Sigmoid, scale=INV_SQRT_D)
                o = outs_pool.tile([PT, T], f16, tag="o")
                nc.vector.tensor_mul(o, gate, pos_sb)
                nc.sync.dma_start(out=out[h, bass.ts(it, PT), :], in_=o)

            def pos_make(it, pool):
                pp = pool.tile([PT, T], f32, tag="ps")
                mm_stripe(pp, 3, it)
                pos_sb = pos_pool.tile([PT, T], f16, tag="pos")
                if it in ACT_CAST_TILES:
                    nc.scalar.activation(pos_sb, pp, Copy)
                else:
                    nc.vector.tensor_copy(pos_sb, pp)
                return pos_sb

            # ---- prologue: pos for tile 0 (cast on ACT; it is idle) ---
            pos_cur = pos_make(0, psA)

            # ---- tiles ------------------------------------------------
            for it in range(NT):
                head(1, it, pos_cur, psB)
                head(0, it, pos_cur, psA)
                pos_next = pos_make(it + 1, psA) if it + 1 < NT else None
                head(2, it, pos_cur, psB)
                if pos_next is not None:
                    pos_cur = pos_next

    nc.finalize()
    return nc


def _get_nc():
    if "nc" not in _NC_CACHE:
        _NC_CACHE["nc"] = _build_nc()
    return _NC_CACHE["nc"]


def kernel(query, key, pos_embed_weight):
    query = np.asarray(query, dtype=np.float32)
    key = np.asarray(key, dtype=np.float32)
    pos_embed_weight = np.asarray(pos_embed_weight, dtype=np.float32)

    q = query.reshape(B * H, T, D)
    k = key.reshape(B * H, T, D)
    # Fold the pos-bias 1/sqrt(D) into the (replicated) P operand: the
    # matmul computes (s*P)(s*P)^T = P P^T / sqrt(D) with s = D**-0.25.
    p_t = np.ascontiguousarray(
        (pos_embed_weight[:T].T * np.float32(D**-0.25)).astype(np.float16)
    )  # [D, T]

    in_maps = []
    for c in range(N_CORES):
        hs = slice(c * HPC, (c + 1) * HPC)
        in_maps.append(
            {
                "qT": np.ascontiguousarray(
                    q[hs].transpose(0, 2, 1).astype(np.float16)
                ),
                "kT": np.ascontiguousarray(
                    k[hs].transpose(0, 2, 1).astype(np.float16)
                ),
                "pT": p_t,
            }
        )

    from concourse.bass_utils import run_bass_kernel_spmd

    nc = _get_nc()
    try:
        res = run_bass_kernel_spmd(
            nc,
            in_maps,
            core_ids=list(range(N_CORES)),
            trace=bool(os.environ.get("KERNEL_TRACE")),
        )
    except Exception:
        # One retry for transient runtime/compile hiccups.
        res = run_bass_kernel_spmd(
            nc, in_maps, core_ids=list(range(N_CORES)), trace=False
        )
    kernel.last_results = res

    full = np.empty((B * H, T, T), dtype=np.float32)
    for c in range(N_CORES):
        full[c * HPC : (c + 1) * HPC] = res.results[c]["out"]
    return full.reshape(B, H, T, T)


kernel.last_results = None


# revision 7
# speedup vs baseline: 1.2090x; 1.2090x over previous
"""CoPEGate Trainium2 kernel.

Computes out[b,h,t,s] = sigmoid((Q K^T)[b,h,t,s] / sqrt(D)) * (P P^T)[t,s] / sqrt(D)
for B=2, H=12, T=2048, D=64 (fp32 in/out), distributed over 8 NeuronCores.

Sharding: the 24 (b,h) pairs are split 3-per-core (head-parallel); the
positional matrix P is replicated and its T x T bias is computed on every
core (reused across that core's 3 heads). No cross-device communication.

Design (all constants HW-measured on this part):

1. fp16 output. The harness tolerance is rel-err 2e-2 (L2); writing the
   output as fp16 (adds ~3e-4 L2 rounding, upcast on host) halves output
   DMA from 48 to 24 MiB/core and moves the bound from HBM writes
   (~147 us floor) to the compute engines.

2. K=128 matmuls. A K=64 [64x128]@[64x512] fp16 chunk matmul streams at
   427 ns (the PE clock governor holds 1.2 GHz for half-array work);
   K=128 runs at ~258 ns (2.4 GHz). All stationary operands are
   zero-padded to 128 contraction rows (zero rows contribute exactly 0):
   lhsT sets [q0;0], [0;q1], [0;q2], [p;0]; the moving tiles pack two
   real operands ([k0;k1] and [p;k2]) so no moving bandwidth is wasted.
   The zero halves are DMA'd from a 256 KiB zero block in DRAM --
   GPSIMD memset was measured to cost a ~16 us startup barrier (IRAM
   load + wait-for-all coalescing), and host-baked zero padding would
   add 3+ MiB to the wire.

3. Half-width stripes. PSUM holds 4 half-stripes [128, 1024] (2 banks
   each, 8 banks total), so matmul->sigmoid round trips (~1.6 us) never
   gate the ACT engine. Full-width ops also measure WORSE per element
   than half-width on both ACT (2360 vs 2x1120 ns) and DVE (1466 vs
   2x675 ns), so every elementwise op is emitted at half width.

Steady-state per row-tile (16 tiles), engine program order:
  PE : s1a s1b s0a s0b pp_a' s2a s2b pp_b'     (2x 512-col chunks each)
  ACT: sig1a sig1b sig0a sig0b sig2a sig2b      (~1120 ns each; bound)
  DVE: mul1a mul1b cast_a' mul0a mul0b cast_b' mul2a mul2b
  DMA: 3x 512 KiB output stripes
The pos stripe for tile it+1 is matmul'd + cast during tile it. Two of
16 casts run on ACT instead of DVE to balance engine totals. GPSIMD is
unused: its tensor ops share SBUF ports with DVE (a concurrent GPSIMD
multiply slowed DVE tensor ops 7x in measurement).

Per-core budget: ACT ~108 us, DVE ~104 us, PE ~66 us, DMA ~27 MiB wire.
Precision: q/k/p fp16 (pos pre-scaled by D**-0.25 on host), fp16 out;
rel err ~5e-4 vs the 2e-2 gate.
"""

import math
import os
import sys

import numpy as np

sys.path.insert(0, "/opt/trn_rl_repo")

B, H, T, D = 2, 12, 2048, 64
N_CORES = 8
HPC = (B * H) // N_CORES  # heads per core
PT = 128  # output row-tile height (SBUF/PSUM partitions)
NT = T // PT  # row tiles
NCHUNK = 512  # matmul moving-operand free dim (one PSUM bank of fp32)
NCH = T // NCHUNK
HW = T // 2  # half-stripe width: [128, HW] f32 = 2 PSUM banks
INV_SQRT_D = 1.0 / math.sqrt(D)

ACT_CAST_TILES = (6, 12)  # tiles whose pos cast runs on ACT (engine balance)

_NC_CACHE = {}


def _build_nc():
    import concourse.bass as bass
    from concourse import bacc, mybir, tile

    f32 = mybir.dt.float32
    f16 = mybir.dt.float16
    Sigmoid = mybir.ActivationFunctionType.Sigmoid
    Copy = mybir.ActivationFunctionType.Copy

    nc = bacc.Bacc("TRN2", target_bir_lowering=False)

    qT = nc.dram_tensor("qT", [HPC, D, T], f16, kind="ExternalInput")
    kT = nc.dram_tensor("kT", [HPC, D, T], f16, kind="ExternalInput")
    # pos scale 1/sqrt(D) is folded into pT on the host (split across both
    # factors), so the pos matmul lands pre-scaled in PSUM.
    pT = nc.dram_tensor("pT", [D, T], f16, kind="ExternalInput")
    zq = nc.dram_tensor("zq", [D, T], f16, kind="ExternalInput")  # zeros
    out = nc.dram_tensor("out", [HPC, T, T], f16, kind="ExternalOutput")

    with tile.TileContext(nc) as tc:
        with tc.tile_pool(name="ins", bufs=1) as ins_pool, \
             tc.tile_pool(name="pos", bufs=3) as pos_pool, \
             tc.tile_pool(name="gate", bufs=6) as gate_pool, \
             tc.tile_pool(name="outs", bufs=12) as outs_pool, \
             tc.tile_pool(name="ps", bufs=4, space="PSUM") as ps_pool:

            # ---- inputs: one [128, T] tile per operand set ------------
            # Moving (rhs): k01 = [k0;k1], pk2 = [p;k2] (both halves real).
            # Stationary (lhsT), zero-padded to K=128 via the zq block:
            #   qz1=[0;q1], qz3=[p;0], qz0=[q0;0], qz2=[0;q2]
            # DMA'd in first-use order (h1 -> pos -> h0 -> h2).
            kT01 = kT[0:2].rearrange("h d t -> (h d) t")
            k01 = ins_pool.tile([2 * D, T], f16, tag="k01")
            nc.sync.dma_start(out=k01, in_=kT01[:, :])
            qz1 = ins_pool.tile([2 * D, T], f16, tag="qz1")
            nc.sync.dma_start(out=qz1[D : 2 * D, :], in_=qT[1][:, :])
            nc.sync.dma_start(out=qz1[0:D, :], in_=zq[:, :])
            pk2 = ins_pool.tile([2 * D, T], f16, tag="pk2")
            nc.sync.dma_start(out=pk2[0:D, :], in_=pT[:, :])
            nc.sync.dma_start(out=pk2[D : 2 * D, :], in_=kT[2][:, :])
            qz3 = ins_pool.tile([2 * D, T], f16, tag="qz3")
            nc.sync.dma_start(out=qz3[0:D, :], in_=pT[:, :])
            nc.sync.dma_start(out=qz3[D : 2 * D, :], in_=zq[:, :])
            qz0 = ins_pool.tile([2 * D, T], f16, tag="qz0")
            nc.sync.dma_start(out=qz0[0:D, :], in_=qT[0][:, :])
            nc.sync.dma_start(out=qz0[D : 2 * D, :], in_=zq[:, :])
            qz2 = ins_pool.tile([2 * D, T], f16, tag="qz2")
            nc.sync.dma_start(out=qz2[D : 2 * D, :], in_=qT[2][:, :])
            nc.sync.dma_start(out=qz2[0:D, :], in_=zq[:, :])

            lhs_t = {0: qz0, 1: qz1, 2: qz2, 3: qz3}
            rhs_t = {0: k01, 1: k01, 2: pk2, 3: pk2}

            def mm_half(psum, s, it, half):
                # Fill one [128, HW] half-stripe = 2 one-bank matmuls.
                lhsT = lhs_t[s][:, bass.ts(it, PT)]
                for jj in range(2):
                    j = 2 * half + jj
                    nc.tensor.matmul(
                        psum[:, bass.ts(jj, NCHUNK)],
                        lhsT,
                        rhs_t[s][:, bass.ts(j, NCHUNK)],
                        start=True,
                        stop=True,
                    )

            def pos_half(pos_sb, it, half):
                # pos half-stripe for tile `it`: matmul + cast f32->f16.
                pp = ps_pool.tile([PT, HW], f32, tag="ps")
                mm_half(pp, 3, it, half)
                dst = pos_sb[:, bass.ts(half, HW)]
                if it in ACT_CAST_TILES:
                    nc.scalar.activation(dst, pp, Copy)
                else:
                    nc.vector.tensor_copy(dst, pp)

            def head_sig(h, it):
                # Score half-stripes + half-width sigmoids -> full gate.
                gate = gate_pool.tile([PT, T], f16, tag="gate")
                for half in range(2):
                    sp = ps_pool.tile([PT, HW], f32, tag="ps")
                    mm_half(sp, h, it, half)
                    nc.scalar.activation(
                        gate[:, bass.ts(half, HW)], sp, Sigmoid, scale=INV_SQRT_D
                    )
                return gate

            def head_mul_dma(h, it, gate, pos_sb):
                o = outs_pool.tile([PT, T], f16, tag="o")
                for half in range(2):
                    hsl = bass.ts(half, HW)
                    nc.vector.tensor_mul(o[:, hsl], gate[:, hsl], pos_sb[:, hsl])
                nc.sync.dma_start(out=out[h, bass.ts(it, PT), :], in_=o)

            # ---- pos prologue for tile 0 (casts on DVE; it is idle) ---
            pos_cur = pos_pool.tile([PT, T], f16, tag="pos")
            for half in range(2):
                pos_half(pos_cur, 0, half)

            # ---- tiles ------------------------------------------------
            # PSUM ring (4 bufs): s1a s1b s0a s0b pp_a' s2a s2b pp_b'
            # -> every sigmoid's refill has >= 850 ns slack; pos for tile
            # it+1 is produced between this tile's muls on DVE.
            for it in range(NT):
                gate1 = head_sig(1, it)
                head_mul_dma(1, it, gate1, pos_cur)

                gate0 = head_sig(0, it)
                pos_next = None
                if it + 1 < NT:
                    pos_next = pos_pool.tile([PT, T], f16, tag="pos")
                    pos_half(pos_next, it + 1, 0)
                head_mul_dma(0, it, gate0, pos_cur)

                gate2 = head_sig(2, it)
                if pos_next is not None:
                    pos_half(pos_next, it + 1, 1)
                head_mul_dma(2, it, gate2, pos_cur)
                if pos_next is not None:
                    pos_cur = pos_next

    nc.finalize()
    return nc


def _get_nc():
    if "nc" not in _NC_CACHE:
        _NC_CACHE["nc"] = _build_nc()
    return _NC_CACHE["nc"]


def kernel(query, key, pos_embed_weight):
    query = np.asarray(query, dtype=np.float32)
    key = np.asarray(key, dtype=np.float32)
    pos_embed_weight = np.asarray(pos_embed_weight, dtype=np.float32)

    q = query.reshape(B * H, T, D)
    k = key.reshape(B * H, T, D)
    # Fold the pos-bias 1/sqrt(D) into the (replicated) P operand: the
    # matmul computes (s*P)(s*P)^T = P P^T / sqrt(D) with s = D**-0.25.
    p_t = np.ascontiguousarray(
        (pos_embed_weight[:T].T * np.float32(D**-0.25)).astype(np.float16)
    )  # [D, T]
    z = np.zeros((D, T), dtype=np.float16)

    in_maps = []
    for c in range(N_CORES):
        hs = slice(c * HPC, (c + 1) * HPC)
        in_maps.append(
            {
                "qT": np.ascontiguousarray(
                    q[hs].transpose(0, 2, 1).astype(np.float16)
                ),
                "kT": np.ascontiguousarray(
                    k[hs].transpose(0, 2, 1).astype(np.float16)
                ),
                "pT": p_t,
                "zq": z,
            }
        )

    from concourse.bass_utils import run_bass_kernel_spmd

    nc = _get_nc()
    try:
        res = run_bass_kernel_spmd(
            nc,
            in_maps,
            core_ids=list(range(N_CORES)),
            trace=bool(os.environ.get("KERNEL_TRACE")),
        )
    except Exception:
        # One retry for transient runtime/compile hiccups.
        res = run_bass_kernel_spmd(
            nc, in_maps, core_ids=list(range(N_CORES)), trace=False
        )
    kernel.last_results = res

    full = np.empty((B * H, T, T), dtype=np.float32)
    for c in range(N_CORES):
        full[c * HPC : (c + 1) * HPC] = res.results[c]["out"]
    return full.reshape(B, H, T, T)


kernel.last_results = None


# revision 8
# speedup vs baseline: 1.3717x; 1.1345x over previous
"""CoPEGate Trainium2 kernel.

Computes out[b,h,t,s] = sigmoid((Q K^T)[b,h,t,s] / sqrt(D)) * (P P^T)[t,s] / sqrt(D)
for B=2, H=12, T=2048, D=64 (fp32 in/out), distributed over 8 NeuronCores.

Sharding: the 24 (b,h) pairs are split 3-per-core (head-parallel); the
positional matrix P is replicated and its T x T bias is computed on every
core (reused across that core's 3 heads). No cross-device communication.

Design (all constants HW-measured on this part):

1. fp16 output. The harness tolerance is rel-err 2e-2 (L2); writing the
   output as fp16 (adds ~3e-4 L2 rounding, upcast on host) halves output
   DMA from 48 to 24 MiB/core and moves the bound from HBM writes
   (~147 us floor) to the ACT engine's sigmoid throughput.

2. K=128 matmuls. A K=64 [64x128]@[64x512] fp16 chunk matmul streams at
   427 ns (the PE clock governor holds 1.2 GHz for half-array work);
   K=128 runs at ~235-258 ns (2.4 GHz). All stationary operands are
   zero-padded to 128 contraction rows on the host (zero rows contribute
   exactly 0): lhsT sets [q0;0], [0;q1], [0;q2], [p;0]; the moving tiles
   pack two real operands each ([k0;k1], [p;k2]) so no moving bandwidth
   is wasted. PE per-core drops from ~109 us to ~60 us.

3. Half-width PSUM stripes [128,1024] (2 banks x 4 buffers = all 8
   banks): a half-stripe's matmul->sigmoid round trip (~1.6 us) never
   gates ACT, which measures wall-to-wall 1087 ns/half-sigmoid (full
   2048-wide stripes in a 2-buffer ring measure ~1.1 us/tile of refill
   bubble, a net loss). Sigmoids are the pacer: 96 x 1087 ~= 104 us.

4. DVE relief: muls run FULL-width (1226 ns vs 2x692 for halves), pos
   casts half-width (subtile-frees PSUM banks early), all on DVE:
   48x1226 + 32x1223 ~= 98 us < ACT. GPSIMD stays idle: its tensor ops
   share SBUF ports with DVE (a concurrent GPSIMD multiply measured a
   7x slowdown of DVE tensor ops).

5. Ramp: inputs arrive as SIX 512 KiB DMAs (QZ[4] lhsT sets + RHS[2]
   moving sets, zeros baked on host) ordered by first use -- v4's 11
   small DMAs serialized ~650 ns each on the SP engine and semaphore
   recycling stretched the ramp to 23 us.

Steady-state per row-tile (16 tiles), engine program order:
  PE : s1a s1b s0a s0b pp_a' s2a s2b pp_b'     (2x 512-col chunks each)
  ACT: sig1a sig1b sig0a sig0b sig2a sig2b     (1087 ns each; pacer)
  DVE: mul1 cast_a' mul0 cast_b' mul2          (pos for tile it+1)
  DMA: 3x 512 KiB output stripes
Precision: q/k/p fp16 (pos pre-scaled by D**-0.25 on host), fp16 out;
rel err ~5e-4 vs the 2e-2 gate.
"""

import math
import os
import sys

import numpy as np

sys.path.insert(0, "/opt/trn_rl_repo")

B, H, T, D = 2, 12, 2048, 64
N_CORES = 8
HPC = (B * H) // N_CORES  # heads per core
PT = 128  # output row-tile height (SBUF/PSUM partitions)
NT = T // PT  # row tiles
NCHUNK = 512  # matmul moving-operand free dim (one PSUM bank of fp32)
NCH = T // NCHUNK
HW = T // 2  # half-stripe width: [128, HW] f32 = 2 PSUM banks
INV_SQRT_D = 1.0 / math.sqrt(D)

_NC_CACHE = {}


def _build_nc():
    import concourse.bass as bass
    from concourse import bacc, mybir, tile

    f32 = mybir.dt.float32
    f16 = mybir.dt.float16
    Sigmoid = mybir.ActivationFunctionType.Sigmoid

    nc = bacc.Bacc("TRN2", target_bir_lowering=False)

    # Host-packed operands (see module docstring):
    #   QZ[0]=[q0;0] QZ[1]=[0;q1] QZ[2]=[0;q2] QZ[3]=[p;0]  (stationary)
    #   RHS[0]=[k0;k1] RHS[1]=[p;k2]                        (moving)
    QZ = nc.dram_tensor("QZ", [4, 2 * D, T], f16, kind="ExternalInput")
    RHS = nc.dram_tensor("RHS", [2, 2 * D, T], f16, kind="ExternalInput")
    out = nc.dram_tensor("out", [HPC, T, T], f16, kind="ExternalOutput")

    with tile.TileContext(nc) as tc:
        with tc.tile_pool(name="ins", bufs=1) as ins_pool, \
             tc.tile_pool(name="pos", bufs=3) as pos_pool, \
             tc.tile_pool(name="gate", bufs=6) as gate_pool, \
             tc.tile_pool(name="outs", bufs=12) as outs_pool, \
             tc.tile_pool(name="ps", bufs=4, space="PSUM") as ps_pool:

            # One [128, T] SBUF tile per operand set, DMA'd in first-use
            # order (head1 scores -> pos prologue -> head0 -> head2).
            qz1 = ins_pool.tile([2 * D, T], f16, tag="qz1")
            nc.sync.dma_start(out=qz1, in_=QZ[1][:, :])
            rk = ins_pool.tile([2 * D, T], f16, tag="rk")
            nc.sync.dma_start(out=rk, in_=RHS[0][:, :])
            qz3 = ins_pool.tile([2 * D, T], f16, tag="qz3")
            nc.sync.dma_start(out=qz3, in_=QZ[3][:, :])
            rp = ins_pool.tile([2 * D, T], f16, tag="rp")
            nc.sync.dma_start(out=rp, in_=RHS[1][:, :])
            qz0 = ins_pool.tile([2 * D, T], f16, tag="qz0")
            nc.sync.dma_start(out=qz0, in_=QZ[0][:, :])
            qz2 = ins_pool.tile([2 * D, T], f16, tag="qz2")
            nc.sync.dma_start(out=qz2, in_=QZ[2][:, :])

            lhs_t = {0: qz0, 1: qz1, 2: qz2, 3: qz3}
            rhs_t = {0: rk, 1: rk, 2: rp, 3: rp}

            def mm_half(psum, s, it, half):
                # Fill one [128, HW] half-stripe = 2 one-bank matmuls.
                lhsT = lhs_t[s][:, bass.ts(it, PT)]
                for jj in range(2):
                    j = 2 * half + jj
                    nc.tensor.matmul(
                        psum[:, bass.ts(jj, NCHUNK)],
                        lhsT,
                        rhs_t[s][:, bass.ts(j, NCHUNK)],
                        start=True,
                        stop=True,
                    )

            def pos_half(pos_sb, it, half):
                # pos half-stripe for tile `it`: matmul + DVE cast f32->f16.
                pp = ps_pool.tile([PT, HW], f32, tag="ps")
                mm_half(pp, 3, it, half)
                nc.vector.tensor_copy(pos_sb[:, bass.ts(half, HW)], pp)

            def head_sig(h, it):
                # Score half-stripes + half-width sigmoids -> full gate.
                gate = gate_pool.tile([PT, T], f16, tag="gate")
                for half in range(2):
                    sp = ps_pool.tile([PT, HW], f32, tag="ps")
                    mm_half(sp, h, it, half)
                    nc.scalar.activation(
                        gate[:, bass.ts(half, HW)], sp, Sigmoid, scale=INV_SQRT_D
                    )
                return gate

            def head_mul_dma(h, it, gate, pos_sb):
                o = outs_pool.tile([PT, T], f16, tag="o")
                nc.vector.tensor_mul(o, gate, pos_sb)
                nc.sync.dma_start(out=out[h, bass.ts(it, PT), :], in_=o)

            # ---- pos prologue for tile 0 (casts on DVE; it is idle) ---
            pos_cur = pos_pool.tile([PT, T], f16, tag="pos")
            for half in range(2):
                pos_half(pos_cur, 0, half)

            # ---- tiles ------------------------------------------------
            # PSUM ring (4 bufs): s1a s1b s0a s0b pp_a' s2a s2b pp_b'
            # -> every sigmoid's refill has >= 850 ns slack; pos for tile
            # it+1 is produced between this tile's muls on DVE.
            for it in range(NT):
                gate1 = head_sig(1, it)
                head_mul_dma(1, it, gate1, pos_cur)

                gate0 = head_sig(0, it)
                pos_next = None
                if it + 1 < NT:
                    pos_next = pos_pool.tile([PT, T], f16, tag="pos")
                    pos_half(pos_next, it + 1, 0)
                head_mul_dma(0, it, gate0, pos_cur)

                gate2 = head_sig(2, it)
                if pos_next is not None:
                    pos_half(pos_next, it + 1, 1)
                head_mul_dma(2, it, gate2, pos_cur)
                if pos_next is not None:
                    pos_cur = pos_next

    nc.finalize()
    return nc


def _get_nc():
    if "nc" not in _NC_CACHE:
        _NC_CACHE["nc"] = _build_nc()
    return _NC_CACHE["nc"]


def kernel(query, key, pos_embed_weight):
    query = np.asarray(query, dtype=np.float32)
    key = np.asarray(key, dtype=np.float32)
    pos_embed_weight = np.asarray(pos_embed_weight, dtype=np.float32)

    q = query.reshape(B * H, T, D)
    k = key.reshape(B * H, T, D)
    # Fold the pos-bias 1/sqrt(D) into the (replicated) P operand: the
    # matmul computes (s*P)(s*P)^T = P P^T / sqrt(D) with s = D**-0.25.
    p_t = (pos_embed_weight[:T].T * np.float32(D**-0.25)).astype(np.float16)

    in_maps = []
    for c in range(N_CORES):
        h0, h1, h2 = c * HPC, c * HPC + 1, c * HPC + 2
        qT = [
            np.ascontiguousarray(q[h].T).astype(np.float16)
            for h in (h0, h1, h2)
        ]
        kT = [
            np.ascontiguousarray(k[h].T).astype(np.float16)
            for h in (h0, h1, h2)
        ]
        qz = np.zeros((4, 2 * D, T), dtype=np.float16)
        qz[0, :D] = qT[0]
        qz[1, D:] = qT[1]
        qz[2, D:] = qT[2]
        qz[3, :D] = p_t
        rhs = np.empty((2, 2 * D, T), dtype=np.float16)
        rhs[0, :D] = kT[0]
        rhs[0, D:] = kT[1]
        rhs[1, :D] = p_t
        rhs[1, D:] = kT[2]
        in_maps.append({"QZ": qz, "RHS": rhs})

    from concourse.bass_utils import run_bass_kernel_spmd

    nc = _get_nc()
    try:
        res = run_bass_kernel_spmd(
            nc,
            in_maps,
            core_ids=list(range(N_CORES)),
            trace=bool(os.environ.get("KERNEL_TRACE")),
        )
    except Exception:
        # One retry for transient runtime/compile hiccups.
        res = run_bass_kernel_spmd(
            nc, in_maps, core_ids=list(range(N_CORES)), trace=False
        )
    kernel.last_results = res

    full = np.empty((B * H, T, T), dtype=np.float32)
    for c in range(N_CORES):
        full[c * HPC : (c + 1) * HPC] = res.results[c]["out"]
    return full.reshape(B, H, T, T)


kernel.last_results = None


# revision 10
# speedup vs baseline: 1.4074x; 1.0260x over previous
"""CoPEGate Trainium2 kernel.

Computes out[b,h,t,s] = sigmoid((Q K^T)[b,h,t,s] / sqrt(D)) * (P P^T)[t,s] / sqrt(D)
for B=2, H=12, T=2048, D=64 (fp32 in/out), distributed over 8 NeuronCores.

Sharding: the 24 (b,h) pairs are split 3-per-core (head-parallel); the
positional matrix P is replicated and its T x T bias is computed on every
core (reused across that core's 3 heads). No cross-device communication.

Design (all constants HW-measured on this part):

1. fp16 output. The harness tolerance is rel-err 2e-2 (L2); writing the
   output as fp16 (adds ~3e-4 L2 rounding, upcast on host) halves output
   DMA from 48 to 24 MiB/core and moves the bound from HBM writes
   (~147 us floor) to the ACT engine's sigmoid throughput.

2. K=128 matmuls. A K=64 [64x128]@[64x512] fp16 chunk matmul streams at
   427 ns (the PE clock governor holds 1.2 GHz for half-array work);
   K=128 runs at ~235-258 ns (2.4 GHz). All stationary operands are
   zero-padded to 128 contraction rows on the host (zero rows contribute
   exactly 0): lhsT sets [q0;0], [0;q1], [0;q2], [p;0]; the moving tiles
   pack two real operands each ([k0;k1], [p;k2]) so no moving bandwidth
   is wasted. PE per-core drops from ~109 us to ~60 us.

3. Half-width PSUM stripes [128,1024] (2 banks x 4 buffers = all 8
   banks): a half-stripe's matmul->sigmoid round trip (~1.6 us) never
   gates ACT, which measures wall-to-wall 1087 ns/half-sigmoid (full
   2048-wide stripes in a 2-buffer ring measure ~1.1 us/tile of refill
   bubble, a net loss). Sigmoids are the pacer: 96 x 1087 ~= 104 us.

4. DVE relief: muls run FULL-width (1226 ns vs 2x692 for halves), pos
   casts half-width (subtile-frees PSUM banks early), all on DVE:
   48x1226 + 32x1223 ~= 98 us < ACT. GPSIMD stays idle: its tensor ops
   share SBUF ports with DVE (a concurrent GPSIMD multiply measured a
   7x slowdown of DVE tensor ops).

5. Ramp: inputs arrive as SIX 512 KiB DMAs (QZ[4] lhsT sets + RHS[2]
   moving sets, zeros baked on host) ordered by first use -- v4's 11
   small DMAs serialized ~650 ns each on the SP engine and semaphore
   recycling stretched the ramp to 23 us.

Steady-state per row-tile (16 tiles), engine program order:
  PE : s1a s1b s0a s0b pp_a' s2a s2b pp_b'     (2x 512-col chunks each)
  ACT: sig1a sig1b sig0a sig0b sig2a sig2b     (1087 ns each; pacer)
  DVE: mul1 cast_a' mul0 cast_b' mul2          (pos for tile it+1)
  DMA: 3x 512 KiB output stripes
Precision: q/k/p fp16 (pos pre-scaled by D**-0.25 on host), fp16 out;
rel err ~5e-4 vs the 2e-2 gate.
"""

import math
import os
import sys

import numpy as np

sys.path.insert(0, "/opt/trn_rl_repo")

B, H, T, D = 2, 12, 2048, 64
N_CORES = 8
HPC = (B * H) // N_CORES  # heads per core
PT = 128  # output row-tile height (SBUF/PSUM partitions)
NT = T // PT  # row tiles
NCHUNK = 512  # matmul moving-operand free dim (one PSUM bank of fp32)
NCH = T // NCHUNK
HW = T // 2  # half-stripe width: [128, HW] f32 = 2 PSUM banks
INV_SQRT_D = 1.0 / math.sqrt(D)

_NC_CACHE = {}


def _build_nc():
    import concourse.bass as bass
    from concourse import bacc, mybir, tile

    f32 = mybir.dt.float32
    f16 = mybir.dt.float16
    Sigmoid = mybir.ActivationFunctionType.Sigmoid

    nc = bacc.Bacc("TRN2", target_bir_lowering=False)

    # Host-packed operands (see module docstring):
    #   QZ[0]=[q0;0] QZ[1]=[0;q1] QZ[2]=[0;q2] QZ[3]=[p;0]  (stationary)
    #   RHS[0]=[k0;k1] RHS[1]=[p;k2]                        (moving)
    QZ = nc.dram_tensor("QZ", [4, 2 * D, T], f16, kind="ExternalInput")
    RHS = nc.dram_tensor("RHS", [2, 2 * D, T], f16, kind="ExternalInput")
    out = nc.dram_tensor("out", [HPC, T, T], f16, kind="ExternalOutput")

    with tile.TileContext(nc) as tc:
        with tc.tile_pool(name="ins", bufs=1) as ins_pool, \
             tc.tile_pool(name="pos", bufs=3) as pos_pool, \
             tc.tile_pool(name="gate", bufs=6) as gate_pool, \
             tc.tile_pool(name="outs", bufs=12) as outs_pool, \
             tc.tile_pool(name="ps", bufs=4, space="PSUM") as ps_pool:

            # One [128, T] SBUF tile per operand set, DMA'd in first-use
            # order (head1 scores -> pos prologue -> head0 -> head2).
            qz1 = ins_pool.tile([2 * D, T], f16, tag="qz1")
            nc.sync.dma_start(out=qz1, in_=QZ[1][:, :])
            rk = ins_pool.tile([2 * D, T], f16, tag="rk")
            nc.sync.dma_start(out=rk, in_=RHS[0][:, :])
            qz3 = ins_pool.tile([2 * D, T], f16, tag="qz3")
            nc.sync.dma_start(out=qz3, in_=QZ[3][:, :])
            rp = ins_pool.tile([2 * D, T], f16, tag="rp")
            nc.sync.dma_start(out=rp, in_=RHS[1][:, :])
            qz0 = ins_pool.tile([2 * D, T], f16, tag="qz0")
            nc.sync.dma_start(out=qz0, in_=QZ[0][:, :])
            qz2 = ins_pool.tile([2 * D, T], f16, tag="qz2")
            nc.sync.dma_start(out=qz2, in_=QZ[2][:, :])

            lhs_t = {0: qz0, 1: qz1, 2: qz2, 3: qz3}
            rhs_t = {0: rk, 1: rk, 2: rp, 3: rp}

            def mm_half(psum, s, it, half):
                # Fill one [128, HW] half-stripe = 2 one-bank matmuls.
                lhsT = lhs_t[s][:, bass.ts(it, PT)]
                for jj in range(2):
                    j = 2 * half + jj
                    nc.tensor.matmul(
                        psum[:, bass.ts(jj, NCHUNK)],
                        lhsT,
                        rhs_t[s][:, bass.ts(j, NCHUNK)],
                        start=True,
                        stop=True,
                    )

            def pos_half(pos_sb, it, half):
                # pos half-stripe for tile `it`: matmul + DVE cast f32->f16.
                pp = ps_pool.tile([PT, HW], f32, tag="ps")
                mm_half(pp, 3, it, half)
                nc.vector.tensor_copy(pos_sb[:, bass.ts(half, HW)], pp)

            def head_sig(h, it):
                # Score half-stripes + half-width sigmoids -> full gate.
                gate = gate_pool.tile([PT, T], f16, tag="gate")
                for half in range(2):
                    sp = ps_pool.tile([PT, HW], f32, tag="ps")
                    mm_half(sp, h, it, half)
                    nc.scalar.activation(
                        gate[:, bass.ts(half, HW)], sp, Sigmoid, scale=INV_SQRT_D
                    )
                return gate

            def head_mul_dma(h, it, gate, pos_sb, split=False):
                o = outs_pool.tile([PT, T], f16, tag="o")
                if split:
                    # Tail trim: half-width muls + DMAs so the final
                    # bytes trail the last sigmoid by ~1.5 us, not ~2.7.
                    for half in range(2):
                        hsl = bass.ts(half, HW)
                        nc.vector.tensor_mul(o[:, hsl], gate[:, hsl], pos_sb[:, hsl])
                        nc.sync.dma_start(
                            out=out[h, bass.ts(it, PT), hsl], in_=o[:, hsl]
                        )
                else:
                    nc.vector.tensor_mul(o, gate, pos_sb)
                    nc.sync.dma_start(out=out[h, bass.ts(it, PT), :], in_=o)

            # ---- tile 0 front: score stripes BEFORE the pos prologue --
            # (qz1/rk are the first DMAs to land; the pos inputs arrive
            # ~2 us later, so filling s1 first starts ACT ~6 us earlier.)
            sp1 = []
            for half in range(2):
                sp = ps_pool.tile([PT, HW], f32, tag="ps")
                mm_half(sp, 1, 0, half)
                sp1.append(sp)
            pos_cur = pos_pool.tile([PT, T], f16, tag="pos")
            for half in range(2):
                pos_half(pos_cur, 0, half)

            # ---- tiles ------------------------------------------------
            # PSUM ring (4 bufs): s1a s1b s0a s0b pp_a' s2a s2b pp_b'
            # -> every sigmoid's refill has >= 850 ns slack; pos for tile
            # it+1 is produced between this tile's muls on DVE.
            for it in range(NT):
                if it == 0:
                    gate1 = gate_pool.tile([PT, T], f16, tag="gate")
                    for half in range(2):
                        nc.scalar.activation(
                            gate1[:, bass.ts(half, HW)], sp1[half],
                            Sigmoid, scale=INV_SQRT_D,
                        )
                else:
                    gate1 = head_sig(1, it)
                head_mul_dma(1, it, gate1, pos_cur)

                gate0 = head_sig(0, it)
                pos_next = None
                if it + 1 < NT:
                    pos_next = pos_pool.tile([PT, T], f16, tag="pos")
                    pos_half(pos_next, it + 1, 0)
                head_mul_dma(0, it, gate0, pos_cur)

                gate2 = head_sig(2, it)
                if pos_next is not None:
                    pos_half(pos_next, it + 1, 1)
                head_mul_dma(2, it, gate2, pos_cur, split=(it == NT - 1))
                if pos_next is not None:
                    pos_cur = pos_next

    nc.finalize()
    return nc


def _get_nc():
    if "nc" not in _NC_CACHE:
        _NC_CACHE["nc"] = _build_nc()
    return _NC_CACHE["nc"]


def kernel(query, key, pos_embed_weight):
    query = np.asarray(query, dtype=np.float32)
    key = np.asarray(key, dtype=np.float32)
    pos_embed_weight = np.asarray(pos_embed_weight, dtype=np.float32)

    q = query.reshape(B * H, T, D)
    k = key.reshape(B * H, T, D)
    # Fold the pos-bias 1/sqrt(D) into the (replicated) P operand: the
    # matmul computes (s*P)(s*P)^T = P P^T / sqrt(D) with s = D**-0.25.
    p_t = (pos_embed_weight[:T].T * np.float32(D**-0.25)).astype(np.float16)

    in_maps = []
    for c in range(N_CORES):
        h0, h1, h2 = c * HPC, c * HPC + 1, c * HPC + 2
        qT = [
            np.ascontiguousarray(q[h].T).astype(np.float16)
            for h in (h0, h1, h2)
        ]
        kT = [
            np.ascontiguousarray(k[h].T).astype(np.float16)
            for h in (h0, h1, h2)
        ]
        qz = np.zeros((4, 2 * D, T), dtype=np.float16)
        qz[0, :D] = qT[0]
        qz[1, D:] = qT[1]
        qz[2, D:] = qT[2]
        qz[3, :D] = p_t
        rhs = np.empty((2, 2 * D, T), dtype=np.float16)
        rhs[0, :D] = kT[0]
        rhs[0, D:] = kT[1]
        rhs[1, :D] = p_t
        rhs[1, D:] = kT[2]
        in_maps.append({"QZ": qz, "RHS": rhs})

    from concourse.bass_utils import run_bass_kernel_spmd

    nc = _get_nc()
    try:
        res = run_bass_kernel_spmd(
            nc,
            in_maps,
            core_ids=list(range(N_CORES)),
            trace=bool(os.environ.get("KERNEL_TRACE")),
        )
    except Exception:
        # One retry for transient runtime/compile hiccups.
        res = run_bass_kernel_spmd(
            nc, in_maps, core_ids=list(range(N_CORES)), trace=False
        )
    kernel.last_results = res

    full = np.empty((B * H, T, T), dtype=np.float32)
    for c in range(N_CORES):
        full[c * HPC : (c + 1) * HPC] = res.results[c]["out"]
    return full.reshape(B, H, T, T)


kernel.last_results = None
